# revision 22
# baseline (speedup 1.0000x reference)
"""Trainium2 Bass kernel: 2-layer GAT (100k nodes, 1.6M edges) on 8 NeuronCores.

Strategy (dst-sharded graph parallel):
  - Nodes dst-sharded contiguously across 8 cores (12500 each), degree-sorted
    within each shard so 128-node tiles have near-uniform in-degree.
  - Per layer, each core computes a "message table" row per owned node:
    [h (128 bf16) | a_src (2 f32, bit-packed)] = 264B rows; one AllGather per
    layer replicates the full 100352-row table to every core (Shared DRAM
    permits only a single writing collective, so chunked AGs are not legal).
  - Edge phase: for each 128-dst tile, slot k gathers each dst's k-th
    non-self incoming edge's source row via indirect DMA (dst == partition).
    Self-loops are handled densely from the local table (no gather).
  - Segment softmax uses exp(leaky_relu(logit)) without max-subtraction
    (logits are O(10), safe in fp32); empty slots point at a phantom row
    whose a_src is -1e4, so exp underflows to zero - no masks needed.
    Denominator gets the reference's +1e-16 so zero-degree rows stay finite.
  - Aggregation: bf16 identity matmuls accumulate alpha-scaled rows in PSUM;
    the self-loop term is one extra matmul over the locally scaled own rows.
"""

import sys

sys.path.insert(0, "/opt/trn_rl_repo")
sys.path.insert(0, "/root/.axon_site/_ro/trn_rl_repo")

import numpy as np
import ml_dtypes

CORES = 8
TILE = 128
ROW = 132  # bf16 elements per table row (264B): [0:128]=h bf16, [128:132]=a_src f32
HID = 64
HEADS = 2
NEG_SLOPE = 0.2
NEG_BIG = -10000.0  # phantom-row a_src: exp(leaky(-1e4)) == 0
EPS = 1e-16
AG_CHUNKS = 1  # Shared DRAM allows a single writing collective per tensor

_RUNNER_CACHE = {}


# ----------------------------------------------------------------------------
# Host-side preprocessing
# ----------------------------------------------------------------------------

def _host_prep(x, edge_index):
    n_nodes = x.shape[0]
    shard = n_nodes // CORES
    ntiles = (shard + TILE - 1) // TILE
    shard_pad = ntiles * TILE
    n_phantom = shard_pad - shard  # zero-degree rows sort to the front

    # Natural edges only: the reference's ADDED self-loop is handled densely
    # on-device; natural i->i edges (if any) stay in the slot grid.
    src = np.asarray(edge_index[0], dtype=np.int64)
    dst = np.asarray(edge_index[1], dtype=np.int64)

    owner = dst // shard

    per_core = []
    deg_by_pos_all = np.zeros((CORES, shard_pad), dtype=np.int64)
    pos_all = np.zeros(n_nodes, dtype=np.int64)  # permuted position of each node
    orders = []

    for c in range(CORES):
        m = owner == c
        s_c = src[m]
        d_loc = dst[m] - c * shard
        deg = np.bincount(d_loc, minlength=shard)
        deg_full = np.concatenate([deg, np.zeros(shard_pad - shard, dtype=deg.dtype)])
        order = np.argsort(deg_full, kind="stable")  # order[j] = local node at pos j
        pos = np.empty(shard_pad, dtype=np.int64)
        pos[order] = np.arange(shard_pad)
        orders.append(order)
        pos_all[c * shard:(c + 1) * shard] = pos[:shard]
        deg_by_pos_all[c] = deg_full[order]
        per_core.append((s_c, d_loc, pos))

    # common K schedule across cores (SPMD: one program)
    K_sched = []
    for t in range(ntiles):
        k = int(deg_by_pos_all[:, t * TILE:(t + 1) * TILE].max())
        K_sched.append(max(k, 1))
    K_arr = np.asarray(K_sched, dtype=np.int64)
    cumK = np.concatenate([[0], np.cumsum(K_arr)])
    SUMK = int(cumK[-1])

    # global table row of node s
    row_map = (np.arange(n_nodes) // shard) * shard_pad + pos_all
    row_map = row_map.astype(np.int32)

    idxs, xTs = [], []
    for c in range(CORES):
        s_c, d_loc, pos = per_core[c]
        p = pos[d_loc]
        ord_e = np.argsort(p, kind="stable")
        p_s = p[ord_e]
        s_s = s_c[ord_e]
        counts = np.bincount(p_s, minlength=shard_pad)
        cum = np.concatenate([[0], np.cumsum(counts)])
        k_slot = np.arange(len(p_s)) - cum[p_s]
        t_of = p_s // TILE
        d_of = p_s % TILE

        # idx image [TILE, SUMK]: partition = dst-within-tile, col = cumK[t]+k
        idx_img = np.zeros((TILE, SUMK), dtype=np.int32)  # empty -> row 0 (phantom)
        idx_img[d_of, cumK[t_of] + k_slot] = row_map[s_s]
        idxs.append(idx_img)

        x_pad = np.zeros((shard_pad, x.shape[1]), dtype=np.float32)
        x_pad[:shard] = x[c * shard:(c + 1) * shard]
        x_perm = x_pad[orders[c]]
        xTs.append(np.ascontiguousarray(x_perm.T).astype(ml_dtypes.bfloat16))

    return {
        "shard": shard,
        "ntiles": ntiles,
        "shard_pad": shard_pad,
        "n_phantom": n_phantom,
        "K_sched": K_sched,
        "SUMK": SUMK,
        "idxs": idxs,
        "xTs": xTs,
        "orders": orders,
    }


def _lift_heads(a2x64):
    """[2, 64] per-head vectors -> [128, 2] block-diagonal lift."""
    out = np.zeros((2 * HID, HEADS), dtype=np.float32)
    for h in range(HEADS):
        out[h * HID:(h + 1) * HID, h] = a2x64[h]
    return out


def _make_weight_inputs(W1, as1, ad1, b1, W2, as2, ad2, b2, lin_w, lin_b):
    bf = ml_dtypes.bfloat16
    W1 = np.asarray(W1, np.float32)
    W2 = np.asarray(W2, np.float32)
    asad1 = np.concatenate([_lift_heads(np.asarray(as1, np.float32)),
                            _lift_heads(np.asarray(ad1, np.float32))], axis=1)  # [128,4]
    asad2_l = np.concatenate([_lift_heads(np.asarray(as2, np.float32)),
                              _lift_heads(np.asarray(ad2, np.float32))], axis=1)  # [128,4]
    wasad2 = np.ascontiguousarray(W2 @ asad2_l)  # [64, 4]
    b1r = np.tile(np.asarray(b1, np.float32)[None, :], (TILE, 1))  # [128, 64]
    b2r = np.tile(np.asarray(b2, np.float32)[None, :], (TILE, 1))
    linw = np.asarray(lin_w, np.float32).reshape(HID, 1)
    linbr = np.tile(np.asarray(lin_b, np.float32).reshape(1, 1), (TILE, 1))  # [128,1]
    return {
        "w1": np.ascontiguousarray(W1).astype(bf),
        "asad1": np.ascontiguousarray(asad1).astype(bf),
        "w2": np.ascontiguousarray(W2).astype(bf),
        "wasad2": np.ascontiguousarray(wasad2).astype(bf),
        "b1r": np.ascontiguousarray(b1r),
        "b2r": np.ascontiguousarray(b2r),
        "linw": np.ascontiguousarray(linw).astype(bf),
        "linbr": np.ascontiguousarray(linbr),
    }


# ----------------------------------------------------------------------------
# Device program
# ----------------------------------------------------------------------------

def _build_program(K_sched, shard_pad, in_ch, n_phantom):
    import concourse.bass as bass
    import concourse.mybir as mybir
    from concourse import bacc
    from concourse.tile import TileContext
    from concourse.masks import make_identity

    f32 = mybir.dt.float32
    bf16 = mybir.dt.bfloat16
    i32 = mybir.dt.int32
    ntiles = len(K_sched)
    SUMK = int(np.sum(K_sched))
    cumK = np.concatenate([[0], np.cumsum(K_sched)])
    tbl_rows = CORES * shard_pad

    # AllGather chunk boundaries (in tiles)
    bounds = [round(g * ntiles / AG_CHUNKS) for g in range(AG_CHUNKS + 1)]

    nc = bacc.Bacc("TRN2", target_bir_lowering=False, debug=False,
                   num_devices=CORES)

    xT = nc.dram_tensor("xT", [in_ch, shard_pad], bf16, kind="ExternalInput")
    idxd = nc.dram_tensor("idx", [TILE, SUMK], i32, kind="ExternalInput")
    w1d = nc.dram_tensor("w1", [in_ch, 2 * HID], bf16, kind="ExternalInput")
    asad1d = nc.dram_tensor("asad1", [2 * HID, 4], bf16, kind="ExternalInput")
    w2d = nc.dram_tensor("w2", [HID, 2 * HID], bf16, kind="ExternalInput")
    wasad2d = nc.dram_tensor("wasad2", [HID, 4], bf16, kind="ExternalInput")
    b1rd = nc.dram_tensor("b1r", [TILE, HID], f32, kind="ExternalInput")
    b2rd = nc.dram_tensor("b2r", [TILE, HID], f32, kind="ExternalInput")
    linwd = nc.dram_tensor("linw", [HID, 1], bf16, kind="ExternalInput")
    linbrd = nc.dram_tensor("linbr", [TILE, 1], f32, kind="ExternalInput")
    yd = nc.dram_tensor("y", [shard_pad, 1], f32, kind="ExternalOutput")

    with TileContext(nc) as tc:
        with (
            tc.tile_pool(name="const", bufs=1) as cpool,
            tc.tile_pool(name="work", bufs=6) as wpool,
            tc.tile_pool(name="gpool", bufs=5) as gpool,
            tc.tile_pool(name="psum", bufs=2, space="PSUM") as ppool,
            tc.tile_pool(name="dram", bufs=1, space="DRAM") as dpool,
        ):
            # ---- persistent tables in HBM
            t1s = dpool.tile([shard_pad, ROW], bf16, tag="t1s")
            t1f = dpool.tile([tbl_rows, ROW], bf16, addr_space="Shared", tag="t1f")
            t2s = dpool.tile([shard_pad, ROW], bf16, tag="t2s")
            t2f = dpool.tile([tbl_rows, ROW], bf16, addr_space="Shared", tag="t2f")

            def ag_chunk(ts, tf, g):
                a, b = bounds[g] * TILE, bounds[g + 1] * TILE
                tf3 = tf[:].rearrange("(c r) w -> c r w", r=shard_pad)
                nc.gpsimd.collective_compute(
                    "AllGather", mybir.AluOpType.bypass,
                    replica_groups=[list(range(CORES))],
                    ins=[ts[a:b, :].opt()], outs=[tf3[:, a:b, :].opt()],
                )

            # ---- constants
            ident = cpool.tile([TILE, TILE], f32, tag="ident")
            make_identity(nc, ident)
            identb = cpool.tile([TILE, TILE], bf16, tag="identb")
            nc.vector.tensor_copy(out=identb[:], in_=ident[:])
            w1_sb = cpool.tile([in_ch, 2 * HID], bf16, tag="w1")
            nc.sync.dma_start(out=w1_sb, in_=w1d[:])
            asad1_sb = cpool.tile([2 * HID, 4], bf16, tag="asad1")
            nc.sync.dma_start(out=asad1_sb, in_=asad1d[:])
            w2_sb = cpool.tile([HID, 2 * HID], bf16, tag="w2")
            nc.sync.dma_start(out=w2_sb, in_=w2d[:])
            wasad2_sb = cpool.tile([HID, 4], bf16, tag="wasad2")
            nc.sync.dma_start(out=wasad2_sb, in_=wasad2d[:])
            b1r_sb = cpool.tile([TILE, HID], f32, tag="b1r")
            nc.sync.dma_start(out=b1r_sb, in_=b1rd[:])
            b2r_sb = cpool.tile([TILE, HID], f32, tag="b2r")
            nc.sync.dma_start(out=b2r_sb, in_=b2rd[:])
            linw_sb = cpool.tile([HID, 1], bf16, tag="linw")
            nc.sync.dma_start(out=linw_sb, in_=linwd[:])
            linbr_sb = cpool.tile([TILE, 1], f32, tag="linbr")
            nc.sync.dma_start(out=linbr_sb, in_=linbrd[:])
            xT_sb = cpool.tile([in_ch, shard_pad], bf16, tag="xT")
            nc.sync.dma_start(out=xT_sb, in_=xT[:])
            idx_sb = cpool.tile([TILE, SUMK], i32, tag="idx")
            nc.sync.dma_start(out=idx_sb, in_=idxd[:])
            aux1_sb = cpool.tile([TILE, ntiles * 4], f32, tag="aux1")
            aux2_sb = cpool.tile([TILE, ntiles * 4], f32, tag="aux2")

            # persistent row stashes (own rows stay in SBUF for self-loop math)
            rows1 = cpool.tile([TILE, ntiles * ROW], bf16, tag="rows1")
            rows2 = cpool.tile([TILE, ntiles * ROW], bf16, tag="rows2")
            # per-layer self-loop weights, computed in one batch
            slf1 = cpool.tile([TILE, ntiles * HEADS], f32, tag="slf1")
            slb1 = cpool.tile([TILE, ntiles * HEADS], bf16, tag="slb1")
            slf2 = cpool.tile([TILE, ntiles * HEADS], f32, tag="slf2")
            slb2 = cpool.tile([TILE, ntiles * HEADS], bf16, tag="slb2")

            def self_weights(aux_sb, slf, slb):
                aux3 = aux_sb[:].rearrange("p (t f) -> p t f", f=4)
                sl3 = slf[:].rearrange("p (t h) -> p t h", h=HEADS)
                nc.vector.tensor_tensor(out=sl3, in0=aux3[:, :, 0:2],
                                        in1=aux3[:, :, 2:4],
                                        op=mybir.AluOpType.add)
                tmp = wpool.tile([TILE, ntiles * HEADS], f32, tag="sltmp")
                nc.vector.tensor_scalar(out=tmp[:], in0=slf[:],
                                        scalar1=NEG_SLOPE, scalar2=None,
                                        op0=mybir.AluOpType.mult)
                nc.vector.tensor_tensor(out=slf[:], in0=slf[:], in1=tmp[:],
                                        op=mybir.AluOpType.max)
                nc.scalar.activation(out=slf[:], in_=slf[:],
                                     func=mybir.ActivationFunctionType.Exp)
                nc.vector.tensor_copy(out=slb[:], in_=slf[:])

            # ---- phase A1: layer-1 table rows
            ag1 = 0
            for t in range(ntiles):
                hT_ps = ppool.tile([TILE, TILE], f32, tag="mm_ps", bufs=4)
                nc.tensor.matmul(out=hT_ps[:], lhsT=w1_sb[:],
                                 rhs=xT_sb[:, t * TILE:(t + 1) * TILE],
                                 start=True, stop=True)
                hT_sb = wpool.tile([TILE, TILE], bf16, tag="hT_sb")
                nc.vector.tensor_copy(out=hT_sb[:], in_=hT_ps[:])
                aux_ps = ppool.tile([TILE, 4], f32, tag="sm_ps", bufs=2)
                nc.tensor.matmul(out=aux_ps[:], lhsT=hT_sb[:], rhs=asad1_sb[:],
                                 start=True, stop=True)
                nc.vector.tensor_copy(out=aux1_sb[:, t * 4:(t + 1) * 4], in_=aux_ps[:])
                h_ps = ppool.tile([TILE, TILE], bf16, tag="mmb_ps", bufs=2)
                nc.tensor.transpose(out=h_ps[:], in_=hT_sb[:], identity=identb[:])
                row_sb = rows1[:, t * ROW:(t + 1) * ROW]
                nc.vector.tensor_copy(out=row_sb[:, 0:TILE], in_=h_ps[:])
                nc.vector.tensor_copy(
                    out=row_sb[:, TILE:TILE + 4].bitcast(f32),
                    in_=aux1_sb[:, t * 4:t * 4 + 2])
                if t == 0:
                    nc.vector.memset(
                        row_sb[0:n_phantom, TILE:TILE + 4].bitcast(f32), NEG_BIG)
                nc.sync.dma_start(out=t1s[t * TILE:(t + 1) * TILE, :], in_=row_sb[:])
                if t + 1 == bounds[ag1 + 1]:
                    ag_chunk(t1s, t1f, ag1)
                    ag1 += 1
            self_weights(aux1_sb, slf1, slb1)

            # ---- edge phase (shared for both layers)
            def edge_phase(t, tbl_full, rows_st, aux_sb, slf, slb_all, br_sb):
                K = K_sched[t]
                cb = int(cumK[t])

                G = gpool.tile([TILE, K * ROW], bf16, tag="G")
                for k in range(K):
                    nc.gpsimd.indirect_dma_start(
                        out=G[:, k * ROW:(k + 1) * ROW], out_offset=None,
                        in_=tbl_full[:],
                        in_offset=bass.IndirectOffsetOnAxis(
                            ap=idx_sb[:, cb + k:cb + k + 1], axis=0))
                G3 = G[:].rearrange("p (k r) -> p k r", r=ROW)
                asrcv = G3[:, :, TILE:TILE + 4].bitcast(f32)  # [TILE, K, 2] f32

                # logits: t = a_src[src] + a_dst[dst]
                tS = wpool.tile([TILE, K * HEADS], f32, tag="tS")
                tS3 = tS[:].rearrange("p (k h) -> p k h", h=HEADS)
                adst = aux_sb[:, t * 4 + 2:t * 4 + 4]
                adst_b = adst.rearrange("p (o h) -> p o h", o=1).to_broadcast(
                    [TILE, K, HEADS])
                nc.vector.tensor_tensor(out=tS3, in0=asrcv,
                                        in1=adst_b, op=mybir.AluOpType.add)
                # ex = exp(leaky_relu(t)); per-head exp also reduces the
                # denominator via accum_out (Scalar engine, frees the DVE)
                tmpL = wpool.tile([TILE, K * HEADS], f32, tag="tmpL")
                nc.vector.tensor_scalar(out=tmpL[:], in0=tS[:],
                                        scalar1=NEG_SLOPE, scalar2=None,
                                        op0=mybir.AluOpType.mult)
                nc.vector.tensor_tensor(out=tS[:], in0=tS[:], in1=tmpL[:],
                                        op=mybir.AluOpType.max)
                denom = wpool.tile([TILE, HEADS], f32, tag="denom")
                tSb = wpool.tile([TILE, K * HEADS], bf16, tag="tSb")
                tSb3 = tSb[:].rearrange("p (k h) -> p k h", h=HEADS)
                for h in range(HEADS):
                    nc.scalar.activation(
                        out=tSb3[:, :, h:h + 1].rearrange("p k o -> p (k o)"),
                        in_=tS3[:, :, h:h + 1].rearrange("p k o -> p (k o)"),
                        func=mybir.ActivationFunctionType.Exp,
                        accum_out=denom[:, h:h + 1])
                nc.vector.tensor_tensor(out=denom[:], in0=denom[:],
                                        in1=slf[:, t * 2:t * 2 + 2],
                                        op=mybir.AluOpType.add)
                # scale gathered h by ex: one op over [TILE, K, 2, 64]
                G4 = G3[:, :, 0:2 * HID].rearrange("p k (h c) -> p k h c",
                                                   h=HEADS)
                exb4 = tSb[:].rearrange("p (k h o) -> p k h o",
                                        h=HEADS, o=1).to_broadcast(
                    [TILE, K, HEADS, HID])
                nc.vector.tensor_tensor(out=G4, in0=G4, in1=exb4,
                                        op=mybir.AluOpType.mult)
                # scale own row by self weight
                own_sb = rows_st[:, t * ROW:t * ROW + 2 * HID]
                own_s = wpool.tile([TILE, 2 * HID], bf16, tag="own_s")
                own4 = own_s[:].rearrange("p (h c) -> p h c", h=HEADS)
                slb4 = slb_all[:, t * 2:t * 2 + 2].rearrange(
                    "p (h o) -> p h o", o=1).to_broadcast([TILE, HEADS, HID])
                nc.vector.tensor_tensor(out=own4,
                                        in0=own_sb.rearrange(
                                            "p (h c) -> p h c", h=HEADS),
                                        in1=slb4, op=mybir.AluOpType.mult)
                # aggregate: PSUM += I @ scaled rows (+ self row)
                out_ps = ppool.tile([TILE, 2 * HID], f32, tag="mm_ps", bufs=4)
                for k in range(K):
                    nc.tensor.matmul(out=out_ps[:], lhsT=identb[:],
                                     rhs=G[:, k * ROW:k * ROW + 2 * HID],
                                     start=(k == 0), stop=False)
                nc.tensor.matmul(out=out_ps[:], lhsT=identb[:],
                                 rhs=own_s[:], start=False, stop=True)
                # mean over heads / (denom+EPS) + bias
                o_sb = wpool.tile([TILE, HID], f32, tag="o_sb")
                o2_sb = wpool.tile([TILE, HID], f32, tag="o2_sb")
                rden = wpool.tile([TILE, HEADS], f32, tag="rden")
                nc.vector.tensor_scalar(out=rden[:], in0=denom[:],
                                        scalar1=EPS, scalar2=None,
                                        op0=mybir.AluOpType.add)
                nc.vector.reciprocal(out=rden[:], in_=rden[:])
                nc.vector.tensor_scalar(out=o_sb[:], in0=out_ps[:, 0:HID],
                                        scalar1=rden[:, 0:1], scalar2=0.5,
                                        op0=mybir.AluOpType.mult,
                                        op1=mybir.AluOpType.mult)
                nc.vector.tensor_scalar(out=o2_sb[:], in0=out_ps[:, HID:2 * HID],
                                        scalar1=rden[:, 1:2], scalar2=0.5,
                                        op0=mybir.AluOpType.mult,
                                        op1=mybir.AluOpType.mult)
                nc.vector.tensor_tensor(out=o_sb[:], in0=o_sb[:], in1=o2_sb[:],
                                        op=mybir.AluOpType.add)
                nc.vector.tensor_tensor(out=o_sb[:], in0=o_sb[:], in1=br_sb[:],
                                        op=mybir.AluOpType.add)
                # ELU: max(x,0)-1 + exp(min(x,0))
                e0 = wpool.tile([TILE, HID], f32, tag="e0")
                nc.vector.tensor_scalar(out=e0[:], in0=o_sb[:], scalar1=0.0,
                                        scalar2=-1.0, op0=mybir.AluOpType.max,
                                        op1=mybir.AluOpType.add)
                e1 = wpool.tile([TILE, HID], f32, tag="e1")
                nc.vector.tensor_scalar(out=e1[:], in0=o_sb[:], scalar1=0.0,
                                        scalar2=None, op0=mybir.AluOpType.min)
                nc.scalar.activation(out=e1[:], in_=e1[:],
                                     func=mybir.ActivationFunctionType.Exp)
                helu = wpool.tile([TILE, HID], bf16, tag="helu")
                nc.vector.tensor_tensor(out=helu[:], in0=e0[:], in1=e1[:],
                                        op=mybir.AluOpType.add)
                return helu

            # ---- B1 + A2 fused
            ag2 = 0
            for t in range(ntiles):
                h2 = edge_phase(t, t1f, rows1, aux1_sb, slf1, slb1, b1r_sb)
                h2T_ps = ppool.tile([HID, TILE], bf16, tag="mmb_ps", bufs=2)
                nc.tensor.transpose(out=h2T_ps[:], in_=h2[:], identity=identb[:])
                h2T_sb = wpool.tile([HID, TILE], bf16, tag="h2T_sb")
                nc.vector.tensor_copy(out=h2T_sb[:], in_=h2T_ps[:])
                hl2_ps = ppool.tile([TILE, 2 * HID], f32, tag="mm_ps", bufs=4)
                nc.tensor.matmul(out=hl2_ps[:], lhsT=h2T_sb[:], rhs=w2_sb[:],
                                 start=True, stop=True)
                aux2_ps = ppool.tile([TILE, 4], f32, tag="sm_ps", bufs=2)
                nc.tensor.matmul(out=aux2_ps[:], lhsT=h2T_sb[:], rhs=wasad2_sb[:],
                                 start=True, stop=True)
                nc.vector.tensor_copy(out=aux2_sb[:, t * 4:(t + 1) * 4],
                                      in_=aux2_ps[:])
                row2_sb = rows2[:, t * ROW:(t + 1) * ROW]
                nc.vector.tensor_copy(out=row2_sb[:, 0:TILE], in_=hl2_ps[:])
                nc.vector.tensor_copy(
                    out=row2_sb[:, TILE:TILE + 4].bitcast(f32),
                    in_=aux2_sb[:, t * 4:t * 4 + 2])
                if t == 0:
                    nc.vector.memset(
                        row2_sb[0:n_phantom, TILE:TILE + 4].bitcast(f32), NEG_BIG)
                nc.sync.dma_start(out=t2s[t * TILE:(t + 1) * TILE, :],
                                  in_=row2_sb[:])
                if t + 1 == bounds[ag2 + 1]:
                    ag_chunk(t2s, t2f, ag2)
                    ag2 += 1
            self_weights(aux2_sb, slf2, slb2)

            # ---- B2 + final linear
            for t in range(ntiles):
                h3 = edge_phase(t, t2f, rows2, aux2_sb, slf2, slb2, b2r_sb)
                h3T_ps = ppool.tile([HID, TILE], bf16, tag="mmb_ps", bufs=2)
                nc.tensor.transpose(out=h3T_ps[:], in_=h3[:], identity=identb[:])
                h3T_sb = wpool.tile([HID, TILE], bf16, tag="h3T_sb")
                nc.vector.tensor_copy(out=h3T_sb[:], in_=h3T_ps[:])
                y_ps = ppool.tile([TILE, 1], f32, tag="sm_ps", bufs=2)
                nc.tensor.matmul(out=y_ps[:], lhsT=h3T_sb[:], rhs=linw_sb[:],
                                 start=True, stop=True)
                y_sb = wpool.tile([TILE, 1], f32, tag="y_sb")
                nc.vector.tensor_tensor(out=y_sb[:], in0=y_ps[:], in1=linbr_sb[:],
                                        op=mybir.AluOpType.add)
                nc.sync.dma_start(out=yd[t * TILE:(t + 1) * TILE, :], in_=y_sb[:])

    nc.compile()
    return nc


# ----------------------------------------------------------------------------
# SPMD execution via PJRT (axon)
# ----------------------------------------------------------------------------

class _SpmdRunner:
    def __init__(self, nc, n_cores):
        import jax
        from jax.sharding import Mesh, PartitionSpec
        from jax.experimental.shard_map import shard_map
        import concourse.mybir as mybir
        from concourse.bass2jax import (_bass_exec_p, partition_id_tensor,
                                        install_neuronx_cc_hook)

        install_neuronx_cc_hook()
        self.jax = jax
        self.n_cores = n_cores
        partition_name = (nc.partition_id_tensor.name
                          if nc.partition_id_tensor else None)
        in_names, out_names, out_avals, zero_outs = [], [], [], []
        for alloc in nc.m.functions[0].allocations:
            if not isinstance(alloc, mybir.MemoryLocationSet):
                continue
            name = alloc.memorylocations[0].name
            if alloc.kind == "ExternalInput":
                if name != partition_name:
                    in_names.append(name)
            elif alloc.kind == "ExternalOutput":
                out_names.append(name)
                shape = tuple(alloc.tensor_shape)
                dtype = mybir.dt.np(alloc.dtype)
                out_avals.append(jax.core.ShapedArray(shape, dtype))
                zero_outs.append(np.zeros(shape, dtype))
        self.in_names, self.out_names = in_names, out_names
        self.out_avals, self.zero_outs = out_avals, zero_outs
        n_params = len(in_names)
        self.n_params = n_params
        all_in_names = list(in_names) + list(out_names)
        if partition_name is not None:
            all_in_names.append(partition_name)

        def _body(*args):
            operands = list(args)
            if partition_name is not None:
                operands.append(partition_id_tensor())
            outs = _bass_exec_p.bind(
                *operands,
                out_avals=tuple(out_avals),
                in_names=tuple(all_in_names),
                out_names=tuple(out_names),
                lowering_input_output_aliases=(),
                sim_require_finite=False,
                sim_require_nnan=False,
                nc=nc,
            )
            return tuple(outs)

        devices = jax.devices()[:n_cores]
        self.mesh = Mesh(np.asarray(devices), ("core",))
        in_specs = (PartitionSpec("core"),) * (n_params + len(out_names))
        out_specs = (PartitionSpec("core"),) * len(out_names)
        self.fn = jax.jit(
            shard_map(_body, mesh=self.mesh, in_specs=in_specs,
                      out_specs=out_specs, check_rep=False),
            keep_unused=True,
        )

    def prep(self, in_maps):
        from jax.sharding import NamedSharding, PartitionSpec
        per_core = [[np.asarray(m[name]) for name in self.in_names]
                    for m in in_maps]
        concat_in = [
            np.concatenate([per_core[c][i] for c in range(self.n_cores)], axis=0)
            for i in range(self.n_params)
        ]
        concat_zeros = [
            np.zeros((self.n_cores * z.shape[0], *z.shape[1:]), z.dtype)
            for z in self.zero_outs
        ]
        sh = NamedSharding(self.mesh, PartitionSpec("core"))
        self.args = [self.jax.device_put(a, sh) for a in (concat_in + concat_zeros)]
        return self

    def run(self):
        outs = self.fn(*self.args)
        self.jax.block_until_ready(outs)
        return [
            {name: np.asarray(outs[i]).reshape(
                self.n_cores, *self.out_avals[i].shape)[c]
             for i, name in enumerate(self.out_names)}
            for c in range(self.n_cores)
        ]


# ----------------------------------------------------------------------------
# Public entry point
# ----------------------------------------------------------------------------

def kernel(x, edge_index, W1, as1, ad1, b1, W2, as2, ad2, b2, lin_w, lin_b):
    x = np.asarray(x, np.float32)
    edge_index = np.asarray(edge_index)
    prep = _host_prep(x, edge_index)
    weights = _make_weight_inputs(W1, as1, ad1, b1, W2, as2, ad2, b2,
                                  lin_w, lin_b)

    key = (tuple(prep["K_sched"]), prep["shard_pad"], x.shape[1])
    if key not in _RUNNER_CACHE:
        nc = _build_program(prep["K_sched"], prep["shard_pad"], x.shape[1],
                            prep["n_phantom"])
        _RUNNER_CACHE[key] = _SpmdRunner(nc, CORES)
    runner = _RUNNER_CACHE[key]

    in_maps = []
    for c in range(CORES):
        m = {"xT": prep["xTs"][c], "idx": prep["idxs"][c]}
        m.update(weights)
        in_maps.append(m)
    runner.prep(in_maps)
    results = runner.run()

    shard = prep["shard"]
    y_full = np.zeros(x.shape[0], dtype=np.float32)
    for c in range(CORES):
        y_c = results[c]["y"][:, 0]
        order = prep["orders"][c]
        real = order < shard
        y_full[c * shard + order[real]] = y_c[real]
    return y_full


# revision 23
# speedup vs baseline: 1.1700x; 1.1700x over previous
"""Trainium2 Bass kernel: 2-layer GAT (100k nodes, 1.6M edges) on 8 NeuronCores.

Strategy (dst-sharded graph parallel):
  - Nodes dst-sharded contiguously across 8 cores (12500 each), degree-sorted
    within each shard so 128-node tiles have near-uniform in-degree.
  - Per layer, each core computes a "message table" row per owned node:
    [h (128 bf16) | a_src (2 f32, bit-packed)] = 264B rows; one AllGather per
    layer replicates the full 100352-row table to every core (Shared DRAM
    permits only a single writing collective, so chunked AGs are not legal).
  - Edge phase: for each 128-dst tile, slot k gathers each dst's k-th
    non-self incoming edge's source row via indirect DMA (dst == partition).
    Self-loops are handled densely from the local table (no gather).
  - Segment softmax uses exp(leaky_relu(logit)) without max-subtraction
    (logits are O(10), safe in fp32); empty slots point at a phantom row
    whose a_src is -1e4, so exp underflows to zero - no masks needed.
    Denominator gets the reference's +1e-16 so zero-degree rows stay finite.
  - Aggregation: bf16 identity matmuls accumulate alpha-scaled rows in PSUM;
    the self-loop term is one extra matmul over the locally scaled own rows.
"""

import sys

sys.path.insert(0, "/opt/trn_rl_repo")
sys.path.insert(0, "/root/.axon_site/_ro/trn_rl_repo")

import numpy as np
import ml_dtypes

CORES = 8
TILE = 128
ROW = 132  # bf16 elements per table row (264B): [0:128]=h bf16, [128:132]=a_src f32
HID = 64
HEADS = 2
NEG_SLOPE = 0.2
NEG_BIG = -10000.0  # phantom-row a_src: exp(leaky(-1e4)) == 0
EPS = 1e-16
AG_CHUNKS = 1  # Shared DRAM allows a single writing collective per tensor

_RUNNER_CACHE = {}


# ----------------------------------------------------------------------------
# Host-side preprocessing
# ----------------------------------------------------------------------------

def _host_prep(x, edge_index):
    n_nodes = x.shape[0]
    shard = n_nodes // CORES
    ntiles = (shard + TILE - 1) // TILE
    shard_pad = ntiles * TILE
    n_phantom = shard_pad - shard  # zero-degree rows sort to the front

    # Natural edges only: the reference's ADDED self-loop is handled densely
    # on-device; natural i->i edges (if any) stay in the slot grid.
    src = np.asarray(edge_index[0], dtype=np.int64)
    dst = np.asarray(edge_index[1], dtype=np.int64)

    owner = dst // shard

    per_core = []
    deg_by_pos_all = np.zeros((CORES, shard_pad), dtype=np.int64)
    pos_all = np.zeros(n_nodes, dtype=np.int64)  # permuted position of each node
    orders = []

    for c in range(CORES):
        m = owner == c
        s_c = src[m]
        d_loc = dst[m] - c * shard
        deg = np.bincount(d_loc, minlength=shard)
        deg_full = np.concatenate([deg, np.zeros(shard_pad - shard, dtype=deg.dtype)])
        order = np.argsort(deg_full, kind="stable")  # order[j] = local node at pos j
        pos = np.empty(shard_pad, dtype=np.int64)
        pos[order] = np.arange(shard_pad)
        orders.append(order)
        pos_all[c * shard:(c + 1) * shard] = pos[:shard]
        deg_by_pos_all[c] = deg_full[order]
        per_core.append((s_c, d_loc, pos))

    # common K schedule across cores (SPMD: one program)
    K_sched = []
    for t in range(ntiles):
        k = int(deg_by_pos_all[:, t * TILE:(t + 1) * TILE].max())
        K_sched.append(max(k, 1))
    K_arr = np.asarray(K_sched, dtype=np.int64)
    cumK = np.concatenate([[0], np.cumsum(K_arr)])
    SUMK = int(cumK[-1])

    # global table row of node s
    row_map = (np.arange(n_nodes) // shard) * shard_pad + pos_all
    row_map = row_map.astype(np.int32)

    idxs, xTs = [], []
    for c in range(CORES):
        s_c, d_loc, pos = per_core[c]
        p = pos[d_loc]
        ord_e = np.argsort(p, kind="stable")
        p_s = p[ord_e]
        s_s = s_c[ord_e]
        counts = np.bincount(p_s, minlength=shard_pad)
        cum = np.concatenate([[0], np.cumsum(counts)])
        k_slot = np.arange(len(p_s)) - cum[p_s]
        t_of = p_s // TILE
        d_of = p_s % TILE

        # idx image [TILE, SUMK]: partition = dst-within-tile, col = cumK[t]+k
        idx_img = np.zeros((TILE, SUMK), dtype=np.int32)  # empty -> row 0 (phantom)
        idx_img[d_of, cumK[t_of] + k_slot] = row_map[s_s]
        idxs.append(idx_img)

        x_pad = np.zeros((shard_pad, x.shape[1]), dtype=np.float32)
        x_pad[:shard] = x[c * shard:(c + 1) * shard]
        x_perm = x_pad[orders[c]]
        xTs.append(np.ascontiguousarray(x_perm.T).astype(ml_dtypes.bfloat16))

    return {
        "shard": shard,
        "ntiles": ntiles,
        "shard_pad": shard_pad,
        "n_phantom": n_phantom,
        "K_sched": K_sched,
        "SUMK": SUMK,
        "idxs": idxs,
        "xTs": xTs,
        "orders": orders,
    }


def _lift_heads(a2x64):
    """[2, 64] per-head vectors -> [128, 2] block-diagonal lift."""
    out = np.zeros((2 * HID, HEADS), dtype=np.float32)
    for h in range(HEADS):
        out[h * HID:(h + 1) * HID, h] = a2x64[h]
    return out


def _make_weight_inputs(W1, as1, ad1, b1, W2, as2, ad2, b2, lin_w, lin_b):
    bf = ml_dtypes.bfloat16
    W1 = np.asarray(W1, np.float32)
    W2 = np.asarray(W2, np.float32)
    asad1 = np.concatenate([_lift_heads(np.asarray(as1, np.float32)),
                            _lift_heads(np.asarray(ad1, np.float32))], axis=1)  # [128,4]
    asad2_l = np.concatenate([_lift_heads(np.asarray(as2, np.float32)),
                              _lift_heads(np.asarray(ad2, np.float32))], axis=1)  # [128,4]
    wasad2 = np.ascontiguousarray(W2 @ asad2_l)  # [64, 4]
    b1r = np.tile(np.asarray(b1, np.float32)[None, :], (TILE, 1))  # [128, 64]
    b2r = np.tile(np.asarray(b2, np.float32)[None, :], (TILE, 1))
    linw = np.asarray(lin_w, np.float32).reshape(HID, 1)
    linbr = np.tile(np.asarray(lin_b, np.float32).reshape(1, 1), (TILE, 1))  # [128,1]
    return {
        "w1": np.ascontiguousarray(W1).astype(bf),
        "asad1": np.ascontiguousarray(asad1).astype(bf),
        "w2": np.ascontiguousarray(W2).astype(bf),
        "wasad2": np.ascontiguousarray(wasad2).astype(bf),
        "b1r": np.ascontiguousarray(b1r),
        "b2r": np.ascontiguousarray(b2r),
        "linw": np.ascontiguousarray(linw).astype(bf),
        "linbr": np.ascontiguousarray(linbr),
    }


# ----------------------------------------------------------------------------
# Device program
# ----------------------------------------------------------------------------

def _build_program(K_sched, shard_pad, in_ch, n_phantom):
    import concourse.bass as bass
    import concourse.mybir as mybir
    from concourse import bacc
    from concourse.tile import TileContext
    from concourse.masks import make_identity

    f32 = mybir.dt.float32
    bf16 = mybir.dt.bfloat16
    i32 = mybir.dt.int32
    ntiles = len(K_sched)
    SUMK = int(np.sum(K_sched))
    cumK = np.concatenate([[0], np.cumsum(K_sched)])
    tbl_rows = CORES * shard_pad

    # AllGather chunk boundaries (in tiles)
    bounds = [round(g * ntiles / AG_CHUNKS) for g in range(AG_CHUNKS + 1)]

    nc = bacc.Bacc("TRN2", target_bir_lowering=False, debug=False,
                   num_devices=CORES)

    xT = nc.dram_tensor("xT", [in_ch, shard_pad], bf16, kind="ExternalInput")
    idxd = nc.dram_tensor("idx", [TILE, SUMK], i32, kind="ExternalInput")
    w1d = nc.dram_tensor("w1", [in_ch, 2 * HID], bf16, kind="ExternalInput")
    asad1d = nc.dram_tensor("asad1", [2 * HID, 4], bf16, kind="ExternalInput")
    w2d = nc.dram_tensor("w2", [HID, 2 * HID], bf16, kind="ExternalInput")
    wasad2d = nc.dram_tensor("wasad2", [HID, 4], bf16, kind="ExternalInput")
    b1rd = nc.dram_tensor("b1r", [TILE, HID], f32, kind="ExternalInput")
    b2rd = nc.dram_tensor("b2r", [TILE, HID], f32, kind="ExternalInput")
    linwd = nc.dram_tensor("linw", [HID, 1], bf16, kind="ExternalInput")
    linbrd = nc.dram_tensor("linbr", [TILE, 1], f32, kind="ExternalInput")
    yd = nc.dram_tensor("y", [shard_pad, 1], f32, kind="ExternalOutput")

    with TileContext(nc) as tc:
        with (
            tc.tile_pool(name="const", bufs=1) as cpool,
            tc.tile_pool(name="work", bufs=6) as wpool,
            tc.tile_pool(name="gpool", bufs=5) as gpool,
            tc.tile_pool(name="psum", bufs=2, space="PSUM") as ppool,
            tc.tile_pool(name="dram", bufs=1, space="DRAM") as dpool,
        ):
            # ---- persistent tables in HBM
            t1s = dpool.tile([shard_pad, ROW], bf16, tag="t1s")
            t1f = dpool.tile([tbl_rows, ROW], bf16, addr_space="Shared", tag="t1f")
            t2s = dpool.tile([shard_pad, ROW], bf16, tag="t2s")
            t2f = dpool.tile([tbl_rows, ROW], bf16, addr_space="Shared", tag="t2f")

            def ag_chunk(ts, tf, g):
                a, b = bounds[g] * TILE, bounds[g + 1] * TILE
                tf3 = tf[:].rearrange("(c r) w -> c r w", r=shard_pad)
                nc.gpsimd.collective_compute(
                    "AllGather", mybir.AluOpType.bypass,
                    replica_groups=[list(range(CORES))],
                    ins=[ts[a:b, :].opt()], outs=[tf3[:, a:b, :].opt()],
                )

            # ---- constants
            ident = cpool.tile([TILE, TILE], f32, tag="ident")
            make_identity(nc, ident)
            identb = cpool.tile([TILE, TILE], bf16, tag="identb")
            nc.vector.tensor_copy(out=identb[:], in_=ident[:])
            w1_sb = cpool.tile([in_ch, 2 * HID], bf16, tag="w1")
            nc.sync.dma_start(out=w1_sb, in_=w1d[:])
            asad1_sb = cpool.tile([2 * HID, 4], bf16, tag="asad1")
            nc.sync.dma_start(out=asad1_sb, in_=asad1d[:])
            w2_sb = cpool.tile([HID, 2 * HID], bf16, tag="w2")
            nc.sync.dma_start(out=w2_sb, in_=w2d[:])
            wasad2_sb = cpool.tile([HID, 4], bf16, tag="wasad2")
            nc.sync.dma_start(out=wasad2_sb, in_=wasad2d[:])
            b1r_sb = cpool.tile([TILE, HID], f32, tag="b1r")
            nc.sync.dma_start(out=b1r_sb, in_=b1rd[:])
            b2r_sb = cpool.tile([TILE, HID], f32, tag="b2r")
            nc.sync.dma_start(out=b2r_sb, in_=b2rd[:])
            linw_sb = cpool.tile([HID, 1], bf16, tag="linw")
            nc.sync.dma_start(out=linw_sb, in_=linwd[:])
            linbr_sb = cpool.tile([TILE, 1], f32, tag="linbr")
            nc.sync.dma_start(out=linbr_sb, in_=linbrd[:])
            xT_sb = cpool.tile([in_ch, shard_pad], bf16, tag="xT")
            nc.sync.dma_start(out=xT_sb, in_=xT[:])
            idx_sb = cpool.tile([TILE, SUMK], i32, tag="idx")
            nc.sync.dma_start(out=idx_sb, in_=idxd[:])
            aux1_sb = cpool.tile([TILE, ntiles * 4], f32, tag="aux1")
            aux2_sb = cpool.tile([TILE, ntiles * 4], f32, tag="aux2")

            # persistent row stashes (own rows stay in SBUF for self-loop math)
            rows1 = cpool.tile([TILE, ntiles * ROW], bf16, tag="rows1")
            rows2 = cpool.tile([TILE, ntiles * ROW], bf16, tag="rows2")
            # per-layer self-loop weights, computed in one batch
            slf1 = cpool.tile([TILE, ntiles * HEADS], f32, tag="slf1")
            slb1 = cpool.tile([TILE, ntiles * HEADS], bf16, tag="slb1")
            slf2 = cpool.tile([TILE, ntiles * HEADS], f32, tag="slf2")
            slb2 = cpool.tile([TILE, ntiles * HEADS], bf16, tag="slb2")

            def self_weights(aux_sb, slf, slb):
                aux3 = aux_sb[:].rearrange("p (t f) -> p t f", f=4)
                sl3 = slf[:].rearrange("p (t h) -> p t h", h=HEADS)
                nc.vector.tensor_tensor(out=sl3, in0=aux3[:, :, 0:2],
                                        in1=aux3[:, :, 2:4],
                                        op=mybir.AluOpType.add)
                tmp = wpool.tile([TILE, ntiles * HEADS], f32, tag="sltmp")
                nc.vector.tensor_scalar(out=tmp[:], in0=slf[:],
                                        scalar1=NEG_SLOPE, scalar2=None,
                                        op0=mybir.AluOpType.mult)
                nc.vector.tensor_tensor(out=slf[:], in0=slf[:], in1=tmp[:],
                                        op=mybir.AluOpType.max)
                nc.scalar.activation(out=slf[:], in_=slf[:],
                                     func=mybir.ActivationFunctionType.Exp)
                nc.vector.tensor_copy(out=slb[:], in_=slf[:])

            # ---- phase A1: layer-1 table rows
            ag1 = 0
            for t in range(ntiles):
                hT_ps = ppool.tile([TILE, TILE], f32, tag="mm_ps", bufs=3)
                nc.tensor.matmul(out=hT_ps[:], lhsT=w1_sb[:],
                                 rhs=xT_sb[:, t * TILE:(t + 1) * TILE],
                                 start=True, stop=True)
                hT_sb = wpool.tile([TILE, TILE], bf16, tag="hT_sb")
                nc.vector.tensor_copy(out=hT_sb[:], in_=hT_ps[:])
                aux_ps = ppool.tile([TILE, 4], f32, tag="sm_ps", bufs=2)
                nc.tensor.matmul(out=aux_ps[:], lhsT=hT_sb[:], rhs=asad1_sb[:],
                                 start=True, stop=True)
                nc.vector.tensor_copy(out=aux1_sb[:, t * 4:(t + 1) * 4], in_=aux_ps[:])
                h_ps = ppool.tile([TILE, TILE], bf16, tag="mmb_ps", bufs=3)
                nc.tensor.transpose(out=h_ps[:], in_=hT_sb[:], identity=identb[:])
                row_sb = rows1[:, t * ROW:(t + 1) * ROW]
                nc.vector.tensor_copy(out=row_sb[:, 0:TILE], in_=h_ps[:])
                nc.vector.tensor_copy(
                    out=row_sb[:, TILE:TILE + 4].bitcast(f32),
                    in_=aux1_sb[:, t * 4:t * 4 + 2])
                if t == 0:
                    nc.vector.memset(
                        row_sb[0:n_phantom, TILE:TILE + 4].bitcast(f32), NEG_BIG)
                nc.sync.dma_start(out=t1s[t * TILE:(t + 1) * TILE, :], in_=row_sb[:])
                if t + 1 == bounds[ag1 + 1]:
                    ag_chunk(t1s, t1f, ag1)
                    ag1 += 1
            self_weights(aux1_sb, slf1, slb1)

            # ---- edge phase (shared for both layers)
            def edge_phase(t, tbl_full, rows_st, aux_sb, slf, slb_all, br_sb):
                K = K_sched[t]
                cb = int(cumK[t])

                G = gpool.tile([TILE, K * ROW], bf16, tag="G")
                for k in range(K):
                    nc.gpsimd.indirect_dma_start(
                        out=G[:, k * ROW:(k + 1) * ROW], out_offset=None,
                        in_=tbl_full[:],
                        in_offset=bass.IndirectOffsetOnAxis(
                            ap=idx_sb[:, cb + k:cb + k + 1], axis=0))
                G3 = G[:].rearrange("p (k r) -> p k r", r=ROW)
                asrcv = G3[:, :, TILE:TILE + 4].bitcast(f32)  # [TILE, K, 2] f32

                # logits: t = a_src[src] + a_dst[dst]
                tS = wpool.tile([TILE, K * HEADS], f32, tag="tS")
                tS3 = tS[:].rearrange("p (k h) -> p k h", h=HEADS)
                adst = aux_sb[:, t * 4 + 2:t * 4 + 4]
                adst_b = adst.rearrange("p (o h) -> p o h", o=1).to_broadcast(
                    [TILE, K, HEADS])
                nc.vector.tensor_tensor(out=tS3, in0=asrcv,
                                        in1=adst_b, op=mybir.AluOpType.add)
                # ex = exp(leaky_relu(t)); per-head exp also reduces the
                # denominator via accum_out (Scalar engine, frees the DVE)
                tmpL = wpool.tile([TILE, K * HEADS], f32, tag="tmpL")
                nc.vector.tensor_scalar(out=tmpL[:], in0=tS[:],
                                        scalar1=NEG_SLOPE, scalar2=None,
                                        op0=mybir.AluOpType.mult)
                nc.vector.tensor_tensor(out=tS[:], in0=tS[:], in1=tmpL[:],
                                        op=mybir.AluOpType.max)
                denom = wpool.tile([TILE, HEADS], f32, tag="denom")
                tSb = wpool.tile([TILE, K * HEADS], bf16, tag="tSb")
                tSb3 = tSb[:].rearrange("p (k h) -> p k h", h=HEADS)
                for h in range(HEADS):
                    nc.scalar.activation(
                        out=tSb3[:, :, h:h + 1].rearrange("p k o -> p (k o)"),
                        in_=tS3[:, :, h:h + 1].rearrange("p k o -> p (k o)"),
                        func=mybir.ActivationFunctionType.Exp,
                        accum_out=denom[:, h:h + 1])
                nc.vector.tensor_tensor(out=denom[:], in0=denom[:],
                                        in1=slf[:, t * 2:t * 2 + 2],
                                        op=mybir.AluOpType.add)
                # scale gathered h by ex: one op over [TILE, K, 2, 64]
                G4 = G3[:, :, 0:2 * HID].rearrange("p k (h c) -> p k h c",
                                                   h=HEADS)
                exb4 = tSb[:].rearrange("p (k h o) -> p k h o",
                                        h=HEADS, o=1).to_broadcast(
                    [TILE, K, HEADS, HID])
                nc.vector.tensor_tensor(out=G4, in0=G4, in1=exb4,
                                        op=mybir.AluOpType.mult)
                # scale own row by self weight
                own_sb = rows_st[:, t * ROW:t * ROW + 2 * HID]
                own_s = wpool.tile([TILE, 2 * HID], bf16, tag="own_s")
                own4 = own_s[:].rearrange("p (h c) -> p h c", h=HEADS)
                slb4 = slb_all[:, t * 2:t * 2 + 2].rearrange(
                    "p (h o) -> p h o", o=1).to_broadcast([TILE, HEADS, HID])
                nc.vector.tensor_tensor(out=own4,
                                        in0=own_sb.rearrange(
                                            "p (h c) -> p h c", h=HEADS),
                                        in1=slb4, op=mybir.AluOpType.mult)
                # aggregate: PSUM += I @ scaled rows (+ self row)
                out_ps = ppool.tile([TILE, 2 * HID], f32, tag="mm_ps", bufs=3)
                for k in range(K):
                    nc.tensor.matmul(out=out_ps[:], lhsT=identb[:],
                                     rhs=G[:, k * ROW:k * ROW + 2 * HID],
                                     start=(k == 0), stop=False)
                nc.tensor.matmul(out=out_ps[:], lhsT=identb[:],
                                 rhs=own_s[:], start=False, stop=True)
                # mean over heads / (denom+EPS) + bias
                o_sb = wpool.tile([TILE, HID], f32, tag="o_sb")
                o2_sb = wpool.tile([TILE, HID], f32, tag="o2_sb")
                rden = wpool.tile([TILE, HEADS], f32, tag="rden")
                nc.vector.tensor_scalar(out=rden[:], in0=denom[:],
                                        scalar1=EPS, scalar2=None,
                                        op0=mybir.AluOpType.add)
                nc.vector.reciprocal(out=rden[:], in_=rden[:])
                nc.vector.tensor_scalar(out=o_sb[:], in0=out_ps[:, 0:HID],
                                        scalar1=rden[:, 0:1], scalar2=0.5,
                                        op0=mybir.AluOpType.mult,
                                        op1=mybir.AluOpType.mult)
                nc.vector.tensor_scalar(out=o2_sb[:], in0=out_ps[:, HID:2 * HID],
                                        scalar1=rden[:, 1:2], scalar2=0.5,
                                        op0=mybir.AluOpType.mult,
                                        op1=mybir.AluOpType.mult)
                nc.vector.tensor_tensor(out=o_sb[:], in0=o_sb[:], in1=o2_sb[:],
                                        op=mybir.AluOpType.add)
                nc.vector.tensor_tensor(out=o_sb[:], in0=o_sb[:], in1=br_sb[:],
                                        op=mybir.AluOpType.add)
                # ELU: max(x,0)-1 + exp(min(x,0))
                e0 = wpool.tile([TILE, HID], f32, tag="e0")
                nc.vector.tensor_scalar(out=e0[:], in0=o_sb[:], scalar1=0.0,
                                        scalar2=-1.0, op0=mybir.AluOpType.max,
                                        op1=mybir.AluOpType.add)
                e1 = wpool.tile([TILE, HID], f32, tag="e1")
                nc.vector.tensor_scalar(out=e1[:], in0=o_sb[:], scalar1=0.0,
                                        scalar2=None, op0=mybir.AluOpType.min)
                nc.scalar.activation(out=e1[:], in_=e1[:],
                                     func=mybir.ActivationFunctionType.Exp)
                helu = wpool.tile([TILE, HID], bf16, tag="helu")
                nc.vector.tensor_tensor(out=helu[:], in0=e0[:], in1=e1[:],
                                        op=mybir.AluOpType.add)
                return helu

            # ---- B1 + A2 fused
            ag2 = 0
            for t in range(ntiles):
                h2 = edge_phase(t, t1f, rows1, aux1_sb, slf1, slb1, b1r_sb)
                h2T_ps = ppool.tile([HID, TILE], bf16, tag="mmb_ps", bufs=3)
                nc.tensor.transpose(out=h2T_ps[:], in_=h2[:], identity=identb[:])
                h2T_sb = wpool.tile([HID, TILE], bf16, tag="h2T_sb")
                nc.vector.tensor_copy(out=h2T_sb[:], in_=h2T_ps[:])
                hl2_ps = ppool.tile([TILE, 2 * HID], f32, tag="mm_ps", bufs=3)
                nc.tensor.matmul(out=hl2_ps[:], lhsT=h2T_sb[:], rhs=w2_sb[:],
                                 start=True, stop=True)
                aux2_ps = ppool.tile([TILE, 4], f32, tag="sm_ps", bufs=2)
                nc.tensor.matmul(out=aux2_ps[:], lhsT=h2T_sb[:], rhs=wasad2_sb[:],
                                 start=True, stop=True)
                nc.vector.tensor_copy(out=aux2_sb[:, t * 4:(t + 1) * 4],
                                      in_=aux2_ps[:])
                row2_sb = rows2[:, t * ROW:(t + 1) * ROW]
                nc.vector.tensor_copy(out=row2_sb[:, 0:TILE], in_=hl2_ps[:])
                nc.vector.tensor_copy(
                    out=row2_sb[:, TILE:TILE + 4].bitcast(f32),
                    in_=aux2_sb[:, t * 4:t * 4 + 2])
                if t == 0:
                    nc.vector.memset(
                        row2_sb[0:n_phantom, TILE:TILE + 4].bitcast(f32), NEG_BIG)
                nc.sync.dma_start(out=t2s[t * TILE:(t + 1) * TILE, :],
                                  in_=row2_sb[:])
                if t + 1 == bounds[ag2 + 1]:
                    ag_chunk(t2s, t2f, ag2)
                    ag2 += 1
            self_weights(aux2_sb, slf2, slb2)

            # ---- B2 + final linear
            for t in range(ntiles):
                h3 = edge_phase(t, t2f, rows2, aux2_sb, slf2, slb2, b2r_sb)
                h3T_ps = ppool.tile([HID, TILE], bf16, tag="mmb_ps", bufs=3)
                nc.tensor.transpose(out=h3T_ps[:], in_=h3[:], identity=identb[:])
                h3T_sb = wpool.tile([HID, TILE], bf16, tag="h3T_sb")
                nc.vector.tensor_copy(out=h3T_sb[:], in_=h3T_ps[:])
                y_ps = ppool.tile([TILE, 1], f32, tag="sm_ps", bufs=2)
                nc.tensor.matmul(out=y_ps[:], lhsT=h3T_sb[:], rhs=linw_sb[:],
                                 start=True, stop=True)
                y_sb = wpool.tile([TILE, 1], f32, tag="y_sb")
                nc.vector.tensor_tensor(out=y_sb[:], in0=y_ps[:], in1=linbr_sb[:],
                                        op=mybir.AluOpType.add)
                nc.sync.dma_start(out=yd[t * TILE:(t + 1) * TILE, :], in_=y_sb[:])

    nc.compile()
    return nc


# ----------------------------------------------------------------------------
# SPMD execution via PJRT (axon)
# ----------------------------------------------------------------------------

class _SpmdRunner:
    def __init__(self, nc, n_cores):
        import jax
        from jax.sharding import Mesh, PartitionSpec
        from jax.experimental.shard_map import shard_map
        import concourse.mybir as mybir
        from concourse.bass2jax import (_bass_exec_p, partition_id_tensor,
                                        install_neuronx_cc_hook)

        install_neuronx_cc_hook()
        self.jax = jax
        self.n_cores = n_cores
        partition_name = (nc.partition_id_tensor.name
                          if nc.partition_id_tensor else None)
        in_names, out_names, out_avals, zero_outs = [], [], [], []
        for alloc in nc.m.functions[0].allocations:
            if not isinstance(alloc, mybir.MemoryLocationSet):
                continue
            name = alloc.memorylocations[0].name
            if alloc.kind == "ExternalInput":
                if name != partition_name:
                    in_names.append(name)
            elif alloc.kind == "ExternalOutput":
                out_names.append(name)
                shape = tuple(alloc.tensor_shape)
                dtype = mybir.dt.np(alloc.dtype)
                out_avals.append(jax.core.ShapedArray(shape, dtype))
                zero_outs.append(np.zeros(shape, dtype))
        self.in_names, self.out_names = in_names, out_names
        self.out_avals, self.zero_outs = out_avals, zero_outs
        n_params = len(in_names)
        self.n_params = n_params
        all_in_names = list(in_names) + list(out_names)
        if partition_name is not None:
            all_in_names.append(partition_name)

        def _body(*args):
            operands = list(args)
            if partition_name is not None:
                operands.append(partition_id_tensor())
            outs = _bass_exec_p.bind(
                *operands,
                out_avals=tuple(out_avals),
                in_names=tuple(all_in_names),
                out_names=tuple(out_names),
                lowering_input_output_aliases=(),
                sim_require_finite=False,
                sim_require_nnan=False,
                nc=nc,
            )
            return tuple(outs)

        devices = jax.devices()[:n_cores]
        self.mesh = Mesh(np.asarray(devices), ("core",))
        in_specs = (PartitionSpec("core"),) * (n_params + len(out_names))
        out_specs = (PartitionSpec("core"),) * len(out_names)
        self.fn = jax.jit(
            shard_map(_body, mesh=self.mesh, in_specs=in_specs,
                      out_specs=out_specs, check_rep=False),
            keep_unused=True,
        )

    def prep(self, in_maps):
        from jax.sharding import NamedSharding, PartitionSpec
        per_core = [[np.asarray(m[name]) for name in self.in_names]
                    for m in in_maps]
        concat_in = [
            np.concatenate([per_core[c][i] for c in range(self.n_cores)], axis=0)
            for i in range(self.n_params)
        ]
        concat_zeros = [
            np.zeros((self.n_cores * z.shape[0], *z.shape[1:]), z.dtype)
            for z in self.zero_outs
        ]
        sh = NamedSharding(self.mesh, PartitionSpec("core"))
        self.args = [self.jax.device_put(a, sh) for a in (concat_in + concat_zeros)]
        return self

    def run(self):
        outs = self.fn(*self.args)
        self.jax.block_until_ready(outs)
        return [
            {name: np.asarray(outs[i]).reshape(
                self.n_cores, *self.out_avals[i].shape)[c]
             for i, name in enumerate(self.out_names)}
            for c in range(self.n_cores)
        ]


# ----------------------------------------------------------------------------
# Public entry point
# ----------------------------------------------------------------------------

def kernel(x, edge_index, W1, as1, ad1, b1, W2, as2, ad2, b2, lin_w, lin_b):
    x = np.asarray(x, np.float32)
    edge_index = np.asarray(edge_index)
    prep = _host_prep(x, edge_index)
    weights = _make_weight_inputs(W1, as1, ad1, b1, W2, as2, ad2, b2,
                                  lin_w, lin_b)

    key = (tuple(prep["K_sched"]), prep["shard_pad"], x.shape[1])
    if key not in _RUNNER_CACHE:
        nc = _build_program(prep["K_sched"], prep["shard_pad"], x.shape[1],
                            prep["n_phantom"])
        _RUNNER_CACHE[key] = _SpmdRunner(nc, CORES)
    runner = _RUNNER_CACHE[key]

    in_maps = []
    for c in range(CORES):
        m = {"xT": prep["xTs"][c], "idx": prep["idxs"][c]}
        m.update(weights)
        in_maps.append(m)
    runner.prep(in_maps)
    results = runner.run()

    shard = prep["shard"]
    y_full = np.zeros(x.shape[0], dtype=np.float32)
    for c in range(CORES):
        y_c = results[c]["y"][:, 0]
        order = prep["orders"][c]
        real = order < shard
        y_full[c * shard + order[real]] = y_c[real]
    return y_full
